# revision 60
# baseline (speedup 1.0000x reference)
"""Multi-head attention (B=2, S=2048, D=1024, H=16) on 8 TRN2 NeuronCores.

Sharding: data-parallel over batch (2 groups of 4 cores) x head-parallel
(4 heads per core). W_q/W_k/W_v are column-sharded by head, W_o is
row-sharded; the 4 partial W_o outputs per batch are summed on the host
(the unshard step), which also undoes the device-side transposed layout.

Per-core kernel v2 - software-pipelined for a gap-free PE stream (the
cost model's p-state ramp makes continuous PE occupancy worth 2x):

  - The PE instruction stream weaves attention blocks (scores -> PV)
    with "filler" matmuls (q/k/v projections of later tiles, output
    projections of earlier tiles) so the exp/mask latency between a
    block's scores and its PV is hidden by independent work.
  - Engine assignment: PE matmuls; Act exp; DVE biases, masks,
    reciprocals, normalizes, PSUM->SBUF moves (gpsimd cannot touch
    PSUM); DMA on the two hardware DGE queues (SP and Act), ordered by
    first-use time.
  - Diagonal causal blocks are trapezoid-trimmed: score matmuls skip
    fully-masked columns (to >=256 free so f32r stays full rate), PV
    matmuls skip further (bf16 moving has no min-size penalty).
  - probs and v are bf16 (mask multiply gets the DVE 2x 16-bit mode);
    scores stay f32r (bf16 q/k would amplify through exp).
  - Softmax denominator rides as a 65th ones-column of v through PV;
    the reciprocal is broadcast to both head-halves with one rank-2 PE
    outer product per (tile, pair).
  - y partials are written bf16 (halves out-DMA; host sums in f64).
"""

import os
from collections import deque

import numpy as np

_B, _S, _D, _H, _DK = 2, 2048, 1024, 16, 64
_HPC = 4          # heads per core
_NCORES = 8
_CPG = 4          # cores per (batch) group
_DPC = _HPC * _DK # 256 projection dims per core
_NEG = -1e9

_program_cache = {}
LAST_RESULTS = None  # BassKernelResults of the most recent run (for profiling)


def _analyze_mask(mask):
    """Classify each [128 k, 512 sq] block of mask^T. Returns (plan, dense).

    plan[i] = tuple of (j, mode, param) for sq-tile i; mode 0 = no mask,
    1 = causal affine_select (param = base), 2 = dense additive mask
    (param = index into dense blocks). Fully-masked blocks are omitted.
    """
    maskT = np.ascontiguousarray(mask.T)
    plan = []
    dense = []
    p_idx = np.arange(128)[:, None]
    s_idx = np.arange(512)[None, :]
    for i in range(_S // 512):
        row = []
        for j in range(_S // 128):
            blk = maskT[j * 128:(j + 1) * 128, i * 512:(i + 1) * 512]
            nz = blk != 0.0
            if nz.all():
                continue  # fully masked: block contributes nothing
            if not nz.any():
                row.append((j, 0, 0))
                continue
            base = i * 512 - j * 128
            causal = (s_idx + i * 512) < (p_idx + j * 128)
            if np.array_equal(nz, causal) and np.all(blk[nz] == 1.0):
                row.append((j, 1, base))
            else:
                row.append((j, 2, len(dense)))
                dense.append(blk * np.float32(_NEG))
        plan.append(tuple(row))
    if dense:
        dense_np = np.stack(dense).astype(np.float32)
    else:
        dense_np = np.zeros((1, 128, 512), np.float32)
    return tuple(plan), dense_np


def _build_program(plan, nblk):
    import concourse.bass as bass  # noqa: F401  (registers engine classes)
    import concourse.tile as tile
    from concourse import bacc, mybir

    F32 = mybir.dt.float32
    F32R = mybir.dt.float32r
    BF16 = mybir.dt.bfloat16
    AF = mybir.ActivationFunctionType
    ALU = mybir.AluOpType
    ts = bass.ts

    nc = bacc.Bacc(None, target_bir_lowering=False, debug=False)

    xq = nc.dram_tensor("xq", [_D, _S], BF16, kind="ExternalInput").ap()
    xk = nc.dram_tensor("xk", [_D, _S], BF16, kind="ExternalInput").ap()
    xv = nc.dram_tensor("xv", [_D, _S], BF16, kind="ExternalInput").ap()
    wq = nc.dram_tensor("wq", [_D, _DPC], BF16, kind="ExternalInput").ap()
    wk = nc.dram_tensor("wk", [_D, _DPC], BF16, kind="ExternalInput").ap()
    wv = nc.dram_tensor("wv", [_D, _DPC], BF16, kind="ExternalInput").ap()
    wo = nc.dram_tensor("wo", [_DPC, _D], F32R, kind="ExternalInput").ap()
    bqk = nc.dram_tensor("bqk", [128, 4], F32, kind="ExternalInput").ap()
    bvb = nc.dram_tensor("bvb", [128, _DPC], F32, kind="ExternalInput").ap()
    mblk = nc.dram_tensor("mblk", [nblk, 128, 512], F32, kind="ExternalInput").ap()
    y = nc.dram_tensor("y", [_D, _S], BF16, kind="ExternalOutput").ap()

    NSEG = len(plan)          # 4 sq tiles
    has_dense = any(m == 2 for row in plan for (_, m, _) in row)

    with tile.TileContext(nc) as tc:
        from contextlib import ExitStack
        with ExitStack() as ctx:
            wpool = ctx.enter_context(tc.tile_pool(name="w", bufs=1))
            cpool = ctx.enter_context(tc.tile_pool(name="const", bufs=1))
            xqkp = ctx.enter_context(tc.tile_pool(name="xqk", bufs=4))
            xvp = ctx.enter_context(tc.tile_pool(name="xv", bufs=2))
            biga = ctx.enter_context(tc.tile_pool(name="biga", bufs=1))
            probp = ctx.enter_context(tc.tile_pool(name="probs", bufs=6))
            recp = ctx.enter_context(tc.tile_pool(name="rec", bufs=3))
            yp = ctx.enter_context(tc.tile_pool(name="y", bufs=8))
            mpool = (
                ctx.enter_context(tc.tile_pool(name="mstream", bufs=3))
                if has_dense and nblk > 2 else None
            )
            mmps = ctx.enter_context(tc.tile_pool(name="mm", bufs=2, space="PSUM"))
            spsp = ctx.enter_context(tc.tile_pool(name="sps", bufs=2, space="PSUM"))
            accp = ctx.enter_context(tc.tile_pool(name="acc", bufs=2, space="PSUM"))

            xq_r = xq.rearrange("(m p) s -> p m s", p=128)
            xk_r = xk.rearrange("(m p) s -> p m s", p=128)
            xv_r = xv.rearrange("(m p) s -> p m s", p=128)
            wq_r = wq.rearrange("(m p) d -> p m d", p=128)
            wk_r = wk.rearrange("(m p) d -> p m d", p=128)
            wv_r = wv.rearrange("(m p) d -> p m d", p=128)

            # ---- input DMAs -------------------------------------------------
            # SP queue: xq/xk/xv column tiles interleaved in consumption
            # order, split in m-halves so the first projection m-loop starts
            # at half the transfer.
            xq_t = [xqkp.tile([128, 8, 512], BF16, tag="xqk", name=f"xq{i}")
                    for i in range(NSEG)]
            xk_t = [xqkp.tile([128, 8, 512], BF16, tag="xqk", name=f"xk{i}")
                    for i in range(NSEG)]
            xv_t = [xvp.tile([128, 8, 512], BF16, tag="xv", name=f"xv{c}")
                    for c in range(NSEG)]
            def x_dma(eng, i, what):
                src, dst = {"q": (xq_r, xq_t), "k": (xk_r, xk_t),
                            "v": (xv_r, xv_t)}[what]
                for lo, hi in ((0, 4), (4, 8)):
                    eng.dma_start(out=dst[i][:, lo:hi, :],
                                  in_=src[:, lo:hi, ts(i, 512)])

            # SP queue: earliest-needed x tiles in consumption order; the
            # very first tile goes in m-quarters so the prologue's first
            # matmul starts as early as possible (larger splits add ~1.7us
            # of per-DMA queue latency, so only tile 0 gets quarters)
            x_dma(nc.sync, 0, "q")
            x_dma(nc.sync, 0, "k")
            x_dma(nc.sync, 0, "v")
            x_dma(nc.sync, 1, "q")
            x_dma(nc.sync, 1, "v")
            x_dma(nc.sync, 2, "q")
            x_dma(nc.sync, 2, "k")
            x_dma(nc.sync, 2, "v")

            # Act queue (2nd hardware DGE): biases + weights first (tiny bq/bk
            # lead so the prologue bias moves never wait), then xk1 (needed
            # ~15us), then the late x tiles.
            wq_sb = wpool.tile([128, 8, _DPC], BF16, tag="wq")
            wk_sb = wpool.tile([128, 8, _DPC], BF16, tag="wk")
            wv_sb = wpool.tile([128, 8, _DPC], BF16, tag="wv")
            wo_sb = wpool.tile([128, 2, _D], F32R, tag="wo")
            for lo, hi in ((0, 4), (4, 8)):
                nc.scalar.dma_start(out=wq_sb[:, lo:hi, :], in_=wq_r[:, lo:hi, :])
            bqk_sb = cpool.tile([128, 4], F32, tag="bqk")
            nc.scalar.dma_start(out=bqk_sb, in_=bqk)
            bq_sb = bqk_sb[:, 0:2]
            bk_sb = bqk_sb[:, 2:4]
            for lo, hi in ((0, 4), (4, 8)):
                nc.scalar.dma_start(out=wk_sb[:, lo:hi, :], in_=wk_r[:, lo:hi, :])
            for lo, hi in ((0, 4), (4, 8)):
                nc.scalar.dma_start(out=wv_sb[:, lo:hi, :], in_=wv_r[:, lo:hi, :])
            x_dma(nc.scalar, 1, "k")
            bvb_sb = cpool.tile([128, _DPC], F32, tag="bvb")
            nc.scalar.dma_start(out=bvb_sb, in_=bvb)
            nc.scalar.dma_start(out=wo_sb,
                                in_=wo.rearrange("(c p) o -> p c o", p=128))
            resident_mask = has_dense and nblk <= 2
            if resident_mask:
                mask_sb = cpool.tile([128, nblk, 512], F32, tag="mask")
                nc.scalar.dma_start(out=mask_sb,
                                    in_=mblk.rearrange("n p s -> p n s"))
            # late x tiles ride the Act DGE queue after the weights
            x_dma(nc.scalar, 3, "q")
            x_dma(nc.scalar, 3, "k")
            x_dma(nc.scalar, 3, "v")

            # ---- small constants -------------------------------------------
            # triangular keep-mask for diagonal blocks: keep iff srel >= p
            m01 = cpool.tile([128, 2, 128], BF16, tag="m01")
            nc.vector.memset(m01, 1.0)
            nc.gpsimd.affine_select(
                out=m01, in_=m01, compare_op=ALU.is_ge, fill=0.0,
                base=0, channel_multiplier=-1, pattern=[[0, 2], [1, 128]],
            )
            # rank-2 selector for the denominator broadcast: contraction row
            # 0 -> out partitions 0-63, row 64 -> out partitions 64-127.
            # (rows 1-63 are zero; partition offsets stay 0/64 so every
            # engine write is legal.)
            psel32 = cpool.tile([65, 128], F32, tag="psel32")
            nc.vector.memset(psel32, 0.0)
            nc.vector.memset(psel32[0:1, 0:64], 1.0)
            nc.vector.memset(psel32[64:65, 64:128], 1.0)
            pair_sel = cpool.tile([65, 128], F32R, tag="psel")
            nc.vector.tensor_copy(pair_sel, psel32)  # f32r memset is illegal

            # ---- big SBUF state --------------------------------------------
            qT = biga.tile([128, 2, _S], F32R, tag="qT")
            kT = biga.tile([128, 2, _S], F32R, tag="kT")
            vsb = biga.tile([128, 16, _HPC * 65], BF16, tag="v")
            attn = biga.tile([128, 2, _S], F32R, tag="attn")
            # ones columns of v (softmax denominator trick)
            nc.vector.memset(
                vsb.rearrange("p sc (h x) -> p sc h x", x=65)[:, :, :, 64:65],
                1.0,
            )

            # ---- filler unit machinery -------------------------------------
            # Each unit is a list of step closures; steps are popped into the
            # PE stream between attention instructions.
            emitted_units = set()

            def qk_unit(which, i, dh):
                """q or k projection of sq tile i, head-pair dh: 8 matmuls in
                4 steps + a bias move."""
                xt = xq_t[i] if which == "q" else xk_t[i]
                w_sb = wq_sb if which == "q" else wk_sb
                b_sb = bq_sb if which == "q" else bk_sb
                dst = qT if which == "q" else kT
                state = {}

                def step(mlo):
                    def run():
                        if mlo == 0:
                            state["ps"] = mmps.tile(
                                [128, 512], F32, tag="mm",
                                name=f"{which}ps{i}{dh}")
                        ps = state["ps"]
                        for m in (mlo, mlo + 1):
                            nc.tensor.matmul(
                                ps, lhsT=w_sb[:, m, ts(dh, 128)],
                                rhs=xt[:, m, :],
                                start=(m == 0), stop=(m == 7),
                            )
                        if mlo == 6:
                            nc.vector.tensor_scalar(
                                dst[:, dh, ts(i, 512)], ps,
                                b_sb[:, dh:dh + 1], None, ALU.add,
                            )
                    return run
                return [step(m) for m in (0, 2, 4, 6)]

            def v_unit(sc):
                """v projection for 128-token chunk sc: 8 matmuls in 4 steps
                + bias add into vsb."""
                col, c = sc // 4, sc % 4
                state = {}

                def step(mlo):
                    def run():
                        if mlo == 0:
                            state["ps"] = mmps.tile(
                                [128, 512], F32, tag="mm", name=f"vps{sc}")
                        ps = state["ps"]
                        for m in (mlo, mlo + 1):
                            nc.tensor.matmul(
                                ps[:, 0:_DPC],
                                lhsT=xv_t[col][:, m, ts(c, 128)],
                                rhs=wv_sb[:, m, :],
                                start=(m == 0), stop=(m == 7),
                            )
                        if mlo == 6:
                            nc.vector.tensor_add(
                                vsb[:, sc, :].rearrange(
                                    "p (h x) -> p h x", x=65)[:, :, 0:64],
                                ps[:, 0:_DPC].rearrange(
                                    "p (h x) -> p h x", x=64),
                                bvb_sb.rearrange("p (h x) -> p h x", x=64),
                            )
                    return run
                return [step(m) for m in (0, 2, 4, 6)]

            def op_unit(i, oc):
                """output projection of sq tile i, 128-row output chunk oc:
                2 matmuls + Pool copy to bf16 + DMA out."""
                def run():
                    yps = mmps.tile([128, 512], F32, tag="mm",
                                    name=f"yps{i}{oc}")
                    for cc in range(2):
                        nc.tensor.matmul(
                            yps, lhsT=wo_sb[:, cc, ts(oc, 128)],
                            rhs=attn[:, cc, ts(i, 512)],
                            start=(cc == 0), stop=(cc == 1),
                        )
                    y_sb = yp.tile([128, 512], BF16, tag="y", name="y_sb")
                    # gpsimd cannot read PSUM; split the moves Act/DVE
                    nc.vector.tensor_copy(y_sb, yps)
                    nc.scalar.dma_start(
                        out=y[oc * 128:(oc + 1) * 128, ts(i, 512)], in_=y_sb)
                return [run]

            fifo = deque()

            # rows of PE work per step, per unit kind (for filler budgeting)
            _STEP_ROWS = {"q": 1024, "k": 1024, "v": 512, "op": 1024}

            def push(key, steps):
                fifo.append({"key": key, "steps": deque(steps),
                             "rows": _STEP_ROWS[key[0]]})

            # FIFO order (deadlines: q/k(i,g) before segment (i,g); v(sc)
            # before the pv that reads it; op(i) after normalize(i,1)).
            for sc in range(4):
                push(("v", sc), v_unit(sc))
            push(("k", 0, 1), qk_unit("k", 0, 1))
            for i in range(1, NSEG):
                push(("q", i, 0), qk_unit("q", i, 0))
                push(("k", i, 0), qk_unit("k", i, 0))
                for sc in range(4 * i, 4 * i + 4):
                    push(("v", sc), v_unit(sc))
                if i >= 2:
                    # hold output projections back one extra tile so the
                    # late (large) segments still have filler work
                    for oc in range(8):
                        push(("op", i - 2, oc), op_unit(i - 2, oc))
                push(("q", i, 1), qk_unit("q", i, 1))
                push(("k", i, 1), qk_unit("k", i, 1))
            for i in (NSEG - 2, NSEG - 1):
                for oc in range(8):
                    push(("op", i, oc), op_unit(i, oc))

            norm_emitted = set()  # (i, g) whose normalize has been emitted

            def blocked(unit):
                key = unit["key"]
                if key[0] == "op":
                    return (key[1], 1) not in norm_emitted
                return False

            def pop_steps(row_budget):
                done = 0
                while done < row_budget:
                    unit = next((u for u in fifo if not blocked(u)), None)
                    if unit is None:
                        return
                    unit["steps"].popleft()()
                    done += unit["rows"]
                    if not unit["steps"]:
                        emitted_units.add(unit["key"])
                        fifo.remove(unit)

            def drain_until(key):
                """Emit FIFO units in order through `key` (deadline drain)."""
                if key in emitted_units:
                    return
                while fifo:
                    unit = fifo[0]
                    while unit["steps"]:
                        unit["steps"].popleft()()
                    emitted_units.add(unit["key"])
                    fifo.popleft()
                    if unit["key"] == key:
                        return
                raise AssertionError(f"unit {key} not found in fifo")

            def drain_only(key):
                """Emit just the unit `key`, leaving earlier units queued."""
                if key in emitted_units:
                    return
                unit = next(u for u in fifo if u["key"] == key)
                while unit["steps"]:
                    unit["steps"].popleft()()
                emitted_units.add(key)
                fifo.remove(unit)

            # ---- prologue: both q projections of tile 0 run before the
            # k projection so the PE stays busy while wk's second half is
            # still in flight on the Act DMA queue
            for st in qk_unit("q", 0, 0):
                st()
            for st in qk_unit("q", 0, 1):
                st()
            for st in qk_unit("k", 0, 0):
                st()
            emitted_units.add(("q", 0, 0))
            emitted_units.add(("q", 0, 1))
            emitted_units.add(("k", 0, 0))

            # ---- attention segments ----------------------------------------
            def emit_scores(i, g, blk):
                """Scores for one block: 2 matmuls -> exp -> (mask). Returns
                the probs tile + trim info for the PV."""
                j, mode, param = blk
                if mode == 1:
                    s0s = min(-param, 256)
                    s0p = -param
                else:
                    s0s = s0p = 0
                sps = spsp.tile([128, 2, 512], F32, tag="sps", name="sps")
                for hh in range(2):
                    nc.tensor.matmul(
                        sps[:, hh, s0s:],
                        lhsT=kT[hh * 64:(hh + 1) * 64, g, ts(j, 128)],
                        rhs=qT[hh * 64:(hh + 1) * 64, g,
                               i * 512 + s0s:(i + 1) * 512],
                        start=True, stop=True,
                    )
                if mode == 2:
                    if resident_mask:
                        mt = mask_sb[:, param, :]
                    else:
                        mt = mpool.tile([128, 512], F32, tag="mtile",
                                        name="mt")
                        nc.sync.dma_start(out=mt, in_=mblk[param])
                    for hh in range(2):
                        nc.vector.tensor_add(sps[:, hh, :], sps[:, hh, :], mt)
                probs = probp.tile([128, 2, 512], BF16, tag="probs",
                                   name="probs")
                # exp only the columns the PV will read ([s0p, 512))
                nc.scalar.activation(probs[:, :, s0p:], sps[:, :, s0p:],
                                     AF.Exp)
                if mode == 1:
                    c0 = -param
                    nc.vector.tensor_mul(
                        probs[:, :, c0:c0 + 128], probs[:, :, c0:c0 + 128],
                        m01,
                    )
                return (j, probs, s0p)

            def make_finisher(i, g, acc):
                # reciprocal -> rank-2 broadcast -> normalize; deferred into
                # the next segment's weave so it never heads the PE queue.
                def fin():
                    rec2 = recp.tile([65, 512], F32R, tag="rec", name="rec2")
                    with nc.allow_low_precision(
                        reason="softmax reciprocal; f32r storage"
                    ):
                        for hh in range(2):
                            nc.vector.reciprocal(rec2[64 * hh:64 * hh + 1, :],
                                                 acc[hh][64:65, :])
                    bc_ps = mmps.tile([128, 512], F32, tag="mm", name="bc_ps")
                    nc.tensor.matmul(bc_ps, lhsT=pair_sel, rhs=rec2,
                                     start=True, stop=True)
                    # ops may read only ONE input from PSUM: stage bc in SBUF
                    bc_sb = recp.tile([128, 512], F32, tag="bcs", name="bc_sb")
                    nc.vector.tensor_copy(bc_sb, bc_ps)
                    for hh in range(2):
                        nc.vector.tensor_mul(
                            attn[hh * 64:(hh + 1) * 64, g, ts(i, 512)],
                            acc[hh][0:64, :],
                            bc_sb[hh * 64:(hh + 1) * 64, :],
                        )
                    norm_emitted.add((i, g))
                return fin

            pending_fin = None
            for i in range(NSEG):
                blocks = plan[i]
                n = len(blocks)
                if n == 0:
                    continue
                for g in range(2):
                    drain_until(("q", i, g))
                    drain_until(("k", i, g))
                    acc = [
                        accp.tile([65, 512], F32, tag="acc",
                                  name=f"acc{i}{g}{hh}")
                        for hh in range(2)
                    ]
                    pending = deque()
                    pending.append(emit_scores(i, g, blocks[0]))
                    if n > 1:
                        pending.append(emit_scores(i, g, blocks[1]))
                    nxt = (i, 1) if g == 0 else (i + 1, 0)
                    # pv lags the scores by 4 blocks: iteration k emits
                    # s(k+2) then pv(k-2), so each block's exp/mask chain
                    # gets ~4 blocks of PE work as latency slack
                    for k in range(n + 2):
                        pop_steps(512)
                        if k == n // 2 and nxt[0] < NSEG:
                            # pull the next segment's q/k projections (and
                            # their DVE bias moves) half a segment early so
                            # its first scores never wait on the DVE queue
                            drain_only(("q",) + nxt)
                            drain_only(("k",) + nxt)
                        if k + 2 < n:
                            pending.append(emit_scores(i, g, blocks[k + 2]))
                        if pending_fin is not None:
                            # previous segment's reciprocal/broadcast/norm:
                            # late enough that its DVE chain is done, but
                            # before pv(0) recycles the acc PSUM banks
                            pending_fin()
                            pending_fin = None
                        if k < 2:
                            continue
                        j, probs, s0p = pending.popleft()
                        drain_only(("v", j))
                        for hh in range(2):
                            h = 2 * g + hh
                            nc.tensor.matmul(
                                acc[hh][:, s0p:],
                                lhsT=vsb[:, j, h * 65:(h + 1) * 65],
                                rhs=probs[:, hh, s0p:],
                                start=(k == 2), stop=(k == n + 1),
                            )
                    pending_fin = make_finisher(i, g, acc)

            # ---- tail: last finisher, then remaining fillers (the last
            # tile's output projection)
            if pending_fin is not None:
                pending_fin()
            while fifo:
                pop_steps(10 ** 9)

    nc.compile()
    return nc


def kernel(**inputs):
    global LAST_RESULTS
    from concourse.bass_utils import run_bass_kernel_spmd

    Q = np.asarray(inputs["Q"], dtype=np.float32)
    K = np.asarray(inputs["K"], dtype=np.float32)
    V = np.asarray(inputs["V"], dtype=np.float32)
    mask = np.asarray(inputs["mask"], dtype=np.float32)
    Wq = np.asarray(inputs["Wq"], dtype=np.float32)
    bq = np.asarray(inputs["bq"], dtype=np.float32)
    Wk = np.asarray(inputs["Wk"], dtype=np.float32)
    bk = np.asarray(inputs["bk"], dtype=np.float32)
    Wv = np.asarray(inputs["Wv"], dtype=np.float32)
    bv = np.asarray(inputs["bv"], dtype=np.float32)
    Wo = np.asarray(inputs["Wo"], dtype=np.float32)
    bo = np.asarray(inputs["bo"], dtype=np.float32)

    plan, dense = _analyze_mask(mask)
    key = (plan, dense.shape[0])
    if key not in _program_cache:
        _program_cache[key] = _build_program(plan, dense.shape[0])
    nc = _program_cache[key]

    import ml_dtypes
    bf16 = ml_dtypes.bfloat16
    sc = np.float32(1.0 / np.sqrt(_DK))
    xqT = [np.ascontiguousarray(Q[b].T).astype(bf16) for b in range(_B)]
    xkT = [np.ascontiguousarray(K[b].T).astype(bf16) for b in range(_B)]
    xvT = [np.ascontiguousarray(V[b].T).astype(bf16) for b in range(_B)]

    in_maps = []
    for core in range(_NCORES):
        b = core // _CPG
        rows = slice((core % _CPG) * _DPC, (core % _CPG) * _DPC + _DPC)
        in_maps.append({
            "xq": xqT[b], "xk": xkT[b], "xv": xvT[b],
            "wq": np.ascontiguousarray((Wq[rows] * sc).T).astype(bf16),
            "wk": np.ascontiguousarray(Wk[rows].T).astype(bf16),
            "wv": np.ascontiguousarray(Wv[rows].T).astype(bf16),
            "wo": np.ascontiguousarray(Wo[:, rows].T),
            "bqk": np.ascontiguousarray(np.concatenate([
                (bq[rows] * sc).reshape(2, 128).T,
                bk[rows].reshape(2, 128).T], axis=1)).astype(np.float32),
            "bvb": np.broadcast_to(bv[rows], (128, _DPC)).copy(),
            "mblk": dense,
        })

    trace = bool(int(os.environ.get("KERNEL_TRACE", "0")))
    LAST_RESULTS = run_bass_kernel_spmd(
        nc, in_maps, list(range(_NCORES)), trace=trace
    )

    out = np.empty((_B, _S, _D), np.float32)
    for b in range(_B):
        acc = np.zeros((_D, _S), np.float64)
        for c in range(_CPG):
            acc += LAST_RESULTS.results[b * _CPG + c]["y"].astype(np.float64)
        out[b] = (acc.T + bo.astype(np.float64)).astype(np.float32)
    return out
